# revision 72
# baseline (speedup 1.0000x reference)
"""Trainium2 Bass kernel for nn_Experts (moe_routing, LoRA-per-expert MLP).

Self-contained: kernel(**inputs) -> np.ndarray [B, S, D] float32.

Strategy: data-parallel over tokens across 8 NeuronCores (512 tokens/core),
base weights replicated. Per core, everything is computed in transposed
activation layout ([feature, token]) so all GEMM operands are natural-layout
SBUF tiles with the contraction on the partition axis:

  router (fp32):  logits[n,e] = x@W_r.T + b_r; softmax via ACT-exp;
                  top-2 via DVE max_with_indices (descending, = top_k order)
  fc1 (bf16):     base1T = W1 @ xT accumulated in PSUM; per-k LoRA correction
                  (z1_k @ B1s) accumulated into the same bank; gelu_tanh+b1
                  fused on ACT
  combine:        hsumT = w0*h0T + w1*h1T  (fc2 is linear in h, so one
                  shared fc2 GEMM instead of two)
  fc2 (bf16):     yT = W2 @ hsumT + sum_k (w_k*z2_k) @ B2s + b2 ⊗ (w0+w1),
                  the bias via a rank-1 matmul accumulated into the same bank

Host side only reshapes/transposes/casts (layout prep), no model math.
"""

import numpy as np
import ml_dtypes

import concourse.bacc as bacc
import concourse.tile as tile
from concourse import mybir
from concourse import bass_utils

BF16 = mybir.dt.bfloat16
F32 = mybir.dt.float32
U32 = mybir.dt.uint32

GELU = mybir.ActivationFunctionType.Gelu_apprx_tanh
EXP = mybir.ActivationFunctionType.Exp
ADD = mybir.AluOpType.add
MULT = mybir.AluOpType.mult
ISEQ = mybir.AluOpType.is_equal

N_CORES = 8


def _emit(nc, tc, a, cfg):
    """Emit the per-core program. `a` maps name -> bass.AP (dram)."""
    import contextlib

    D, F, E, R, NT = cfg["D"], cfg["F"], cfg["E"], cfg["R"], cfg["NT"]
    DC, FB = D // 128, F // 128  # d-chunks, f-blocks
    DB = D // 128                # output d-blocks
    NB = NT // 128               # router n-blocks
    ER = E * R

    dbg = cfg.get("debug", False)

    def dump(name, tile_ap):
        if dbg and name in a:
            nc.sync.dma_start(out=a[name], in_=tile_ap)

    with contextlib.ExitStack() as ctx:
        ec = ctx.enter_context
        const = ec(tc.tile_pool(name="const", bufs=1))
        xf32p = ec(tc.tile_pool(name="xf32p", bufs=3))
        w1p = ec(tc.tile_pool(name="w1p", bufs=4))
        w2p = ec(tc.tile_pool(name="w2p", bufs=2))
        work = ec(tc.tile_pool(name="work", bufs=2))
        hwork = ec(tc.tile_pool(name="hwork", bufs=4))
        pA = ec(tc.tile_pool(name="pA", bufs=4, space="PSUM"))
        pC = ec(tc.tile_pool(name="pC", bufs=2, space="PSUM"))
        pU = ec(tc.tile_pool(name="pU", bufs=1, space="PSUM"))

        # ---- DMA staging: router weight + x tiles first (router starts
        # ~immediately; bf16 x is derived on-chip per-tile), then W1 blocks.
        wrt = const.tile([128, DC, E], F32, tag="wrt")
        nc.sync.dma_start(out=wrt, in_=a["wrt"])
        xfs = []
        for dc in range(DC):
            xf = xf32p.tile([128, NT], F32, tag="xf", name=f"xf{dc}", bufs=5)
            nc.sync.dma_start(out=xf, in_=a["xt"][:, dc, :])
            xfs.append(xf)
        xbf = const.tile([128, DC, NT], BF16, tag="xbf")
        W1_PRE = 4
        w1sbs = {}
        for fb in range(W1_PRE):
            w1sbs[fb] = w1p.tile(
                [128, DC, 128], BF16, tag="w1", name=f"w1pre{fb}"
            )
            nc.sync.dma_start(out=w1sbs[fb], in_=a["w1t"][fb])

        hsum = const.tile([128, FB, NT], BF16, tag="hsum")

        # ---- router logits (fp32) + on-chip bf16 cast of x ----
        lg_ps = pC.tile([128, NB, E], F32, tag="c")
        for dc in range(DC):
            xf = xfs[dc]
            nc.scalar.copy(out=xbf[:, dc, :], in_=xf)
            for nb in range(NB):
                # start=True clears the WHOLE psum bank, so only the very
                # first matmul into this bank may set it.
                nc.tensor.matmul(
                    lg_ps[:, nb, :],
                    xf[:, nb * 128:(nb + 1) * 128],
                    wrt[:, dc, :],
                    start=(dc == 0 and nb == 0),
                    stop=False,
                )

        # ---- remaining resident constants (needed later than the router) ----
        ident = const.tile([128, 128], F32, tag="ident")
        nc.sync.dma_start(out=ident, in_=a["ident"])
        ones = const.tile([1, 128], F32, tag="ones")
        nc.sync.dma_start(out=ones, in_=a["ones"])
        onesb = const.tile([1, 128], BF16, tag="onesb")
        nc.sync.dma_start(out=onesb, in_=a["onesb"])
        eoer = const.tile([ER, 1], F32, tag="eoer")
        nc.sync.dma_start(out=eoer, in_=a["eoer"])
        b1r = const.tile([128, FB], F32, tag="b1r")
        nc.sync.dma_start(out=b1r, in_=a["b1r"])
        brow = const.tile([1, E], F32, tag="brow")
        nc.sync.dma_start(out=brow, in_=a["brow"])
        a1t = const.tile([128, DC, ER], BF16, tag="a1t")
        nc.sync.dma_start(out=a1t, in_=a["a1t"])
        a2t = const.tile([128, FB, ER], BF16, tag="a2t")
        nc.sync.dma_start(out=a2t, in_=a["a2t"])
        b1s = const.tile([ER, F], BF16, tag="b1s")
        nc.sync.dma_start(out=b1s, in_=a["b1s"])
        b2s = const.tile([ER, D], BF16, tag="b2s")
        nc.sync.dma_start(out=b2s, in_=a["b2s"])
        b2t = const.tile([1, D], BF16, tag="b2t")
        nc.sync.dma_start(out=b2t, in_=a["b2t"])
        for nb in range(NB):  # + ones ⊗ b_r
            nc.tensor.matmul(
                lg_ps[:, nb, :], ones, brow, start=False, stop=True
            )

        # ---- softmax denominators + top-2 ----
        el = const.tile([128, NB, E], F32, tag="el")
        nc.scalar.activation(out=el, in_=lg_ps, func=EXP)
        ssum = const.tile([128, NB], F32, tag="ssum")
        nc.vector.reduce_sum(out=ssum, in_=el, axis=mybir.AxisListType.X)
        rs = const.tile([128, NB], F32, tag="rs")
        nc.vector.reciprocal(out=rs, in_=ssum)
        lgsb = const.tile([128, NB, E], F32, tag="lgsb")
        nc.vector.tensor_copy(out=lgsb, in_=lg_ps)
        mx = const.tile([128, NB, 8], F32, tag="mx")
        mi = const.tile([128, NB, 8], U32, tag="mi")
        for nb in range(NB):
            nc.vector.max_with_indices(mx[:, nb, :], mi[:, nb, :], lgsb[:, nb, :])
        dump("d_lgsb", lgsb)
        dump("d_mx", mx)
        dump("d_mi", mi)
        dump("d_el", el)
        dump("d_ssum", ssum)
        wiv = const.tile([128, NB, 4], F32, tag="wiv")
        ev = const.tile([128, NB, 2], F32, tag="ev")
        nc.scalar.activation(out=ev, in_=mx[:, :, 0:2], func=EXP)
        for nb in range(NB):
            nc.vector.tensor_scalar_mul(
                wiv[:, nb, 0:2], ev[:, nb, :], rs[:, nb:nb + 1]
            )
        nc.vector.tensor_copy(out=wiv[:, :, 2:4], in_=mi[:, :, 0:2])

        # ---- transpose w0, w1, i0, i1 each to its own [1, NT] row tile ----
        rows = []  # w0T, w1T, i0T, i1T (bf16: w is bf16 downstream anyway,
        # idx values 0..7 are exact)
        for j in range(4):
            rt = const.tile([1, NT], BF16, tag=f"rowT{j}")
            for nb in range(NB):
                tps = pC.tile([1, 128], F32, tag="c")
                nc.tensor.transpose(tps, wiv[:, nb, j:j + 1], ident)
                nc.vector.tensor_copy(
                    out=rt[:, nb * 128:(nb + 1) * 128], in_=tps
                )
            rows.append(rt)
            dump(f"d_row{j}", rt)
        dump("d_wiv", wiv)

        z2w0 = const.tile([ER, NT], BF16, tag="z2w0")
        z2w1 = const.tile([ER, NT], BF16, tag="z2w1")
        swT = const.tile([1, NT], BF16, tag="swT")
        nc.vector.tensor_tensor(out=swT, in0=rows[0], in1=rows[1], op=ADD)

        # ---- expert masks ohT_k [ER, NT] and weight rows wrep_k [128, NT] ----
        oh = []
        wrep = []
        for k in range(2):
            rep_ps = pC.tile([ER, NT], F32, tag="c")
            nc.tensor.matmul(
                rep_ps, onesb[0:1, 0:ER], rows[2 + k], start=True, stop=True
            )
            ohk = const.tile([ER, NT], BF16, tag=f"oh{k}")
            nc.vector.tensor_scalar(
                out=ohk, in0=rep_ps, scalar1=eoer, scalar2=None, op0=ISEQ
            )
            oh.append(ohk)
            dump(f"d_oh{k}", ohk)
            wr_ps = pC.tile([128, NT], F32, tag="c")
            nc.tensor.matmul(wr_ps, onesb, rows[k], start=True, stop=True)
            wrk = const.tile([128, NT], BF16, tag=f"wrep{k}")
            nc.vector.tensor_copy(out=wrk, in_=wr_ps)
            wrep.append(wrk)
            dump(f"d_wrep{k}", wrk)

        # ---- u1 = A1_flat @ xT, masked -> z1_k ----
        u1_ps = pU.tile([ER, NT], F32, tag="u0")
        for dc in range(DC):
            nc.tensor.matmul(
                u1_ps, a1t[:, dc, :], xbf[:, dc, :],
                start=(dc == 0), stop=(dc == DC - 1),
            )
        if dbg:
            du1 = const.tile([ER, NT], F32, tag="du1")
            nc.vector.tensor_copy(out=du1, in_=u1_ps)
            dump("d_u1", du1)
        z1_0 = const.tile([ER, NT], BF16, tag="z1_0")
        nc.vector.tensor_tensor(out=z1_0, in0=u1_ps, in1=oh[0], op=MULT)
        dump("d_z10", z1_0)
        # delta mask: z1d = u1 * (oh1 - oh0); base+c0+B1s@z1d == base+c1
        ohd = const.tile([ER, NT], BF16, tag="ohd")
        nc.vector.tensor_tensor(
            out=ohd, in0=oh[1], in1=oh[0], op=mybir.AluOpType.subtract
        )
        z1d = const.tile([ER, NT], BF16, tag="z1d")
        nc.vector.tensor_tensor(out=z1d, in0=u1_ps, in1=ohd, op=MULT)

        # ---- fc1 loop over f-block PAIRS ----
        # Per pair: 32 base matmuls back-to-back, then ONE cluster of small
        # (partial-array) matmuls: c0 for this pair, the z1d expert-delta for
        # the previous pair, and u2 for the pair before that. Clustering the
        # row_grp/col_grp matmuls halves full/partial LDWEIGHTS transitions.
        u20_ps = pU.tile([ER, NT], F32, tag="u0")
        u21_ps = pU.tile([ER, NT], F32, tag="u1")
        pend_c1d = []  # [(fb, fbs, bank, h0)] awaiting the delta half
        pend_u2 = []   # [(fb, h0, h1)] awaiting u2/hsum

        def flush_u2():
            for fbq, h0q, h1q in pend_u2:
                nc.tensor.matmul(
                    u20_ps, a2t[:, fbq, :], h0q,
                    start=(fbq == 0), stop=(fbq == FB - 1),
                )
                nc.tensor.matmul(
                    u21_ps, a2t[:, fbq, :], h1q,
                    start=(fbq == 0), stop=(fbq == FB - 1),
                )
            for fbq, h0q, h1q in pend_u2:
                t0 = work.tile([128, NT], BF16, tag="t0")
                nc.vector.tensor_tensor(out=t0, in0=h0q, in1=wrep[0], op=MULT)
                t1 = work.tile([128, NT], BF16, tag="t1")
                nc.vector.tensor_tensor(out=t1, in0=h1q, in1=wrep[1], op=MULT)
                nc.vector.tensor_tensor(
                    out=hsum[:, fbq, :], in0=t0, in1=t1, op=ADD
                )
            pend_u2.clear()

        def flush_c1d():
            done = []
            for fbq, fbsq, bank, h0q in pend_c1d:
                nc.tensor.matmul(
                    bank, b1s[:, fbsq], z1d, start=False, stop=True,
                    skip_group_check=True,
                )
                h1q = hwork.tile([128, NT], BF16, tag="h1", bufs=4)
                nc.scalar.activation(
                    out=h1q, in_=bank, func=GELU, bias=b1r[:, fbq:fbq + 1]
                )
                done.append((fbq, h0q, h1q))
            pend_c1d.clear()
            return done

        for p in range(FB // 2):
            newc1d = []
            for fb in (2 * p, 2 * p + 1):
                fbs = slice(fb * 128, (fb + 1) * 128)
                if fb in w1sbs:
                    w1sb = w1sbs.pop(fb)
                else:
                    w1sb = w1p.tile([128, DC, 128], BF16, tag="w1")
                    nc.sync.dma_start(out=w1sb, in_=a["w1t"][fb])
                base_ps = pA.tile([128, NT], F32, tag="a")
                for dc in range(DC):
                    nc.tensor.matmul(
                        base_ps, w1sb[:, dc, :], xbf[:, dc, :],
                        start=(dc == 0), stop=False,
                    )
                newc1d.append((fb, fbs, base_ps))
            # small-matmul cluster: c0 for this pair first (gates gelu-h0)
            augmented = []
            for fb, fbs, bank in newc1d:
                nc.tensor.matmul(
                    bank, b1s[:, fbs], z1_0, start=False, stop=True,
                    skip_group_check=True,
                )
                h0 = hwork.tile([128, NT], BF16, tag="h0", bufs=6)
                nc.scalar.activation(
                    out=h0, in_=bank, func=GELU, bias=b1r[:, fb:fb + 1]
                )
                augmented.append((fb, fbs, bank, h0))
            done = flush_c1d()
            flush_u2()
            pend_u2.extend(done)
            pend_c1d.extend(augmented)
        pend_u2.extend(flush_c1d())
        flush_u2()
        dump("d_hsum", hsum)

        # ---- z2w_k = u2_k * oh_k * w_k ----
        for k, u2_ps in ((0, u20_ps), (1, u21_ps)):
            if dbg:
                du2 = const.tile([ER, NT], F32, tag=f"du2{k}")
                nc.vector.tensor_copy(out=du2, in_=u2_ps)
                dump(f"d_u2{k}", du2)
            tz = work.tile([ER, NT], F32, tag="tz")
            nc.vector.tensor_tensor(out=tz, in0=u2_ps, in1=oh[k], op=MULT)
            dst = z2w0 if k == 0 else z2w1
            nc.vector.tensor_tensor(out=dst, in0=tz, in1=wrep[k][0:ER, :], op=MULT)
            dump(f"d_z2w{k}", dst)

        # ---- fc2 loop over d-block QUADS (small matmuls clustered) ----
        FH = FB // 2  # load W2 per d-block in two halves
        for dp in range(DB // 4):
            items = []
            for db in range(4 * dp, 4 * dp + 4):
                dbs = slice(db * 128, (db + 1) * 128)
                w2h = []
                for h in range(2):
                    t = w2p.tile([128, FH, 128], BF16, tag="w2")
                    nc.sync.dma_start(
                        out=t, in_=a["w2t"][db][:, h * FH:(h + 1) * FH, :]
                    )
                    w2h.append(t)
                y_ps = pA.tile([128, NT], F32, tag="a")
                for fc in range(FB):
                    nc.tensor.matmul(
                        y_ps, w2h[fc // FH][:, fc % FH, :], hsum[:, fc, :],
                        start=(fc == 0), stop=False,
                    )
                items.append((dbs, y_ps))
            for dbs, y_ps in items:
                nc.tensor.matmul(
                    y_ps, b2s[:, dbs], z2w0, start=False, stop=False
                )
                nc.tensor.matmul(
                    y_ps, b2s[:, dbs], z2w1, start=False, stop=False
                )
                nc.tensor.matmul(y_ps, b2t[:, dbs], swT, start=False, stop=True)
            for dbs, y_ps in items:
                osb = work.tile([128, NT], F32, tag="osb")
                nc.vector.tensor_copy(out=osb, in_=y_ps)
                nc.sync.dma_start(out=a["rest"][dbs, :], in_=osb)


def _build(cfg):
    nc = bacc.Bacc("TRN2", target_bir_lowering=False, debug=False)
    D, F, E, R, NT = cfg["D"], cfg["F"], cfg["E"], cfg["R"], cfg["NT"]
    DC, FB = D // 128, F // 128
    ER = E * R

    def din(name, shape, dt):
        return nc.dram_tensor(name, shape, dt, kind="ExternalInput").ap()

    a = {
        "xt": din("xt", [128, DC, NT], F32),
        "xb": din("xb", [128, DC, NT], BF16),
        "w1t": din("w1t", [FB, 128, DC, 128], BF16),
        "w2t": din("w2t", [DC, 128, FB, 128], BF16),
        "wrt": din("wrt", [128, DC, E], F32),
        "brow": din("brow", [1, E], F32),
        "a1t": din("a1t", [128, DC, ER], BF16),
        "a2t": din("a2t", [128, FB, ER], BF16),
        "b1s": din("b1s", [ER, F], BF16),
        "b2s": din("b2s", [ER, D], BF16),
        "b2t": din("b2t", [1, D], BF16),
        "b1r": din("b1r", [128, FB], F32),
        "ident": din("ident", [128, 128], F32),
        "ones": din("ones", [1, 128], F32),
        "onesb": din("onesb", [1, 128], BF16),
        "eoer": din("eoer", [ER, 1], F32),
        "rest": nc.dram_tensor("rest", [D, NT], F32, kind="ExternalOutput").ap(),
    }
    if cfg.get("debug", False):
        def dout(name, shape, dt):
            a[name] = nc.dram_tensor(name, shape, dt, kind="ExternalOutput").ap()

        NB = NT // 128
        for j in range(4):
            dout(f"d_row{j}", [1, NT], F32)
        dout("d_lgsb", [128, NB, E], F32)
        dout("d_mx", [128, NB, 8], F32)
        dout("d_mi", [128, NB, 8], U32)
        dout("d_el", [128, NB, E], F32)
        dout("d_ssum", [128, NB], F32)
        dout("d_wiv", [128, NB, 4], F32)
        for k in range(2):
            dout(f"d_oh{k}", [ER, NT], BF16)
            dout(f"d_wrep{k}", [128, NT], F32)
            dout(f"d_z1{k}", [ER, NT], BF16)
            dout(f"d_u2{k}", [ER, NT], F32)
            dout(f"d_z2w{k}", [ER, NT], BF16)
        dout("d_u1", [ER, NT], F32)
        dout("d_base0", [128, NT], F32)
        dout("d_h0", [128, NT], BF16)
        dout("d_h1", [128, NT], BF16)
        dout("d_hsum", [128, FB, NT], BF16)
    with tile.TileContext(nc) as tc:
        _emit(nc, tc, a, cfg)
    nc.compile()
    return nc


def _host_prep(hidden_states, W_r, b_r, W1, b1, W2, b2, A1, B1, A2, B2):
    """Layout prep on host. Returns (in_maps, cfg)."""
    x = np.ascontiguousarray(np.asarray(hidden_states, dtype=np.float32))
    B_, S_, D = x.shape
    N = B_ * S_
    NT = N // N_CORES
    F = np.asarray(W1).shape[0]
    E, R = np.asarray(A1).shape[0], np.asarray(A1).shape[1]
    ER = E * R
    DC, FB = D // 128, F // 128
    SCALING = 2.0

    W_r = np.asarray(W_r, dtype=np.float32)
    W1 = np.asarray(W1, dtype=np.float32)
    W2 = np.asarray(W2, dtype=np.float32)
    A1 = np.asarray(A1, dtype=np.float32)
    B1 = np.asarray(B1, dtype=np.float32)
    A2 = np.asarray(A2, dtype=np.float32)
    B2 = np.asarray(B2, dtype=np.float32)
    b_r = np.asarray(b_r, dtype=np.float32)
    b1 = np.asarray(b1, dtype=np.float32)
    b2 = np.asarray(b2, dtype=np.float32)

    bf = ml_dtypes.bfloat16
    w1t = np.ascontiguousarray(
        W1.reshape(FB, 128, DC, 128).transpose(0, 3, 2, 1)
    ).astype(bf)
    w2t = np.ascontiguousarray(
        W2.reshape(DC, 128, FB, 128).transpose(0, 3, 2, 1)
    ).astype(bf)
    wrt = np.ascontiguousarray(W_r.T.reshape(DC, 128, E).transpose(1, 0, 2))
    a1t = np.ascontiguousarray(
        A1.reshape(ER, D).T.reshape(DC, 128, ER).transpose(1, 0, 2)
    ).astype(bf)
    a2t = np.ascontiguousarray(
        A2.reshape(ER, F).T.reshape(FB, 128, ER).transpose(1, 0, 2)
    ).astype(bf)
    b1s = (B1.transpose(0, 2, 1).reshape(ER, F) * SCALING).astype(bf)
    b2s = (B2.transpose(0, 2, 1).reshape(ER, D) * SCALING).astype(bf)
    b2t = b2[None, :].astype(bf)
    b1r = np.ascontiguousarray(b1.reshape(FB, 128).T)
    shared = {
        "w1t": w1t,
        "w2t": w2t,
        "wrt": wrt,
        "brow": np.ascontiguousarray(b_r[None, :]),
        "a1t": a1t,
        "a2t": a2t,
        "b1s": b1s,
        "b2s": b2s,
        "b2t": b2t,
        "b1r": b1r,
        "ident": np.eye(128, dtype=np.float32),
        "ones": np.ones((1, 128), dtype=np.float32),
        "onesb": np.ones((1, 128), dtype=bf),
        "eoer": (np.arange(ER, dtype=np.float32) // R).reshape(ER, 1),
    }
    xf = x.reshape(N, D)
    in_maps = []
    for c in range(N_CORES):
        xc = xf[c * NT:(c + 1) * NT]
        xt = np.ascontiguousarray(xc.reshape(NT, DC, 128).transpose(2, 1, 0))
        in_maps.append({"xt": xt, "xb": xt.astype(bf), **shared})
    cfg = {"D": D, "F": F, "E": E, "R": R, "NT": NT}
    return in_maps, cfg, (B_, S_, N)


_nc_cache = {}


def _run(inputs, trace=False, trace_cores=None, debug=False):
    in_maps, cfg, (B_, S_, N) = _host_prep(**inputs)
    if debug:
        cfg["debug"] = True
    key = tuple(sorted(cfg.items()))
    if key not in _nc_cache:
        _nc_cache[key] = _build(cfg)
    nc = _nc_cache[key]
    res = bass_utils.run_bass_kernel_spmd(
        nc,
        in_maps,
        core_ids=list(range(N_CORES)),
        trace=trace,
        trace_cores=trace_cores,
    )
    D, NT = cfg["D"], cfg["NT"]
    out = np.empty((N, D), dtype=np.float32)
    for c in range(N_CORES):
        out[c * NT:(c + 1) * NT] = res.results[c]["rest"].T
    return out.reshape(B_, S_, D), res


def kernel(**inputs):
    out, _ = _run(inputs)
    return out


# revision 75
# speedup vs baseline: 1.0093x; 1.0093x over previous
"""Trainium2 Bass kernel for nn_Experts (moe_routing, LoRA-per-expert MLP).

Self-contained: kernel(**inputs) -> np.ndarray [B, S, D] float32.

Strategy: data-parallel over tokens across 8 NeuronCores (512 tokens/core),
base weights replicated. Per core, everything is computed in transposed
activation layout ([feature, token]) so all GEMM operands are natural-layout
SBUF tiles with the contraction on the partition axis:

  router (fp32):  logits[n,e] = x@W_r.T + b_r; softmax via ACT-exp;
                  top-2 via DVE max_with_indices (descending, = top_k order)
  fc1 (bf16):     base1T = W1 @ xT accumulated in PSUM; per-k LoRA correction
                  (z1_k @ B1s) accumulated into the same bank; gelu_tanh+b1
                  fused on ACT
  combine:        hsumT = w0*h0T + w1*h1T  (fc2 is linear in h, so one
                  shared fc2 GEMM instead of two)
  fc2 (bf16):     yT = W2 @ hsumT + sum_k (w_k*z2_k) @ B2s + b2 ⊗ (w0+w1),
                  the bias via a rank-1 matmul accumulated into the same bank

Host side only reshapes/transposes/casts (layout prep), no model math.
"""

import numpy as np
import ml_dtypes

import concourse.bacc as bacc
import concourse.tile as tile
from concourse import mybir
from concourse import bass_utils

BF16 = mybir.dt.bfloat16
F32 = mybir.dt.float32
U32 = mybir.dt.uint32

GELU = mybir.ActivationFunctionType.Gelu_apprx_tanh
EXP = mybir.ActivationFunctionType.Exp
ADD = mybir.AluOpType.add
MULT = mybir.AluOpType.mult
ISEQ = mybir.AluOpType.is_equal

N_CORES = 8


def _emit(nc, tc, a, cfg):
    """Emit the per-core program. `a` maps name -> bass.AP (dram)."""
    import contextlib

    D, F, E, R, NT = cfg["D"], cfg["F"], cfg["E"], cfg["R"], cfg["NT"]
    DC, FB = D // 128, F // 128  # d-chunks, f-blocks
    DB = D // 128                # output d-blocks
    NB = NT // 128               # router n-blocks
    ER = E * R

    dbg = cfg.get("debug", False)

    def dump(name, tile_ap):
        if dbg and name in a:
            nc.sync.dma_start(out=a[name], in_=tile_ap)

    with contextlib.ExitStack() as ctx:
        ec = ctx.enter_context
        const = ec(tc.tile_pool(name="const", bufs=1))
        xf32p = ec(tc.tile_pool(name="xf32p", bufs=3))
        w1p = ec(tc.tile_pool(name="w1p", bufs=3))
        w2p = ec(tc.tile_pool(name="w2p", bufs=2))
        work = ec(tc.tile_pool(name="work", bufs=2))
        hwork = ec(tc.tile_pool(name="hwork", bufs=4))
        pA = ec(tc.tile_pool(name="pA", bufs=4, space="PSUM"))
        pC = ec(tc.tile_pool(name="pC", bufs=2, space="PSUM"))
        pU = ec(tc.tile_pool(name="pU", bufs=1, space="PSUM"))

        # ---- DMA staging: router weight + x tiles first (router starts
        # ~immediately; bf16 x is derived on-chip per-tile), then W1 blocks.
        wrt = const.tile([128, DC, E], F32, tag="wrt")
        nc.sync.dma_start(out=wrt, in_=a["wrt"])
        xfs = []
        for dc in range(DC):
            xf = xf32p.tile([128, NT], F32, tag="xf", name=f"xf{dc}", bufs=5)
            nc.sync.dma_start(out=xf, in_=a["xt"][:, dc, :])
            xfs.append(xf)
        xbf = const.tile([128, DC, NT], BF16, tag="xbf")
        W1_PRE = 3
        w1sbs = {}
        for fb in range(W1_PRE):
            w1sbs[fb] = w1p.tile(
                [128, DC, 128], BF16, tag="w1", name=f"w1pre{fb}"
            )
            nc.sync.dma_start(out=w1sbs[fb], in_=a["w1t"][fb])

        hsum = const.tile([128, FB, NT], BF16, tag="hsum")

        # ---- router logits (fp32) + on-chip bf16 cast of x ----
        lg_ps = pC.tile([128, NB, E], F32, tag="c")
        for dc in range(DC):
            xf = xfs[dc]
            nc.scalar.copy(out=xbf[:, dc, :], in_=xf)
            for nb in range(NB):
                # start=True clears the WHOLE psum bank, so only the very
                # first matmul into this bank may set it.
                nc.tensor.matmul(
                    lg_ps[:, nb, :],
                    xf[:, nb * 128:(nb + 1) * 128],
                    wrt[:, dc, :],
                    start=(dc == 0 and nb == 0),
                    stop=False,
                )

        # ---- remaining resident constants (needed later than the router) ----
        ident = const.tile([128, 128], F32, tag="ident")
        nc.sync.dma_start(out=ident, in_=a["ident"])
        ones = const.tile([1, 128], F32, tag="ones")
        nc.sync.dma_start(out=ones, in_=a["ones"])
        onesb = const.tile([1, 128], BF16, tag="onesb")
        nc.sync.dma_start(out=onesb, in_=a["onesb"])
        eoer = const.tile([ER, 1], F32, tag="eoer")
        nc.sync.dma_start(out=eoer, in_=a["eoer"])
        b1r = const.tile([128, FB], F32, tag="b1r")
        nc.sync.dma_start(out=b1r, in_=a["b1r"])
        brow = const.tile([1, E], F32, tag="brow")
        nc.sync.dma_start(out=brow, in_=a["brow"])
        a1t = const.tile([128, DC, ER], BF16, tag="a1t")
        nc.sync.dma_start(out=a1t, in_=a["a1t"])
        a2t = const.tile([128, FB, ER], BF16, tag="a2t")
        nc.sync.dma_start(out=a2t, in_=a["a2t"])
        b1s = const.tile([ER, F], BF16, tag="b1s")
        nc.sync.dma_start(out=b1s, in_=a["b1s"])
        b2s = const.tile([ER, D], BF16, tag="b2s")
        nc.sync.dma_start(out=b2s, in_=a["b2s"])
        b2t = const.tile([1, D], BF16, tag="b2t")
        nc.sync.dma_start(out=b2t, in_=a["b2t"])
        for nb in range(NB):  # + ones ⊗ b_r
            nc.tensor.matmul(
                lg_ps[:, nb, :], ones, brow, start=False, stop=True
            )

        # ---- softmax denominators + top-2 ----
        el = const.tile([128, NB, E], F32, tag="el")
        nc.scalar.activation(out=el, in_=lg_ps, func=EXP)
        ssum = const.tile([128, NB], F32, tag="ssum")
        nc.vector.reduce_sum(out=ssum, in_=el, axis=mybir.AxisListType.X)
        rs = const.tile([128, NB], F32, tag="rs")
        nc.vector.reciprocal(out=rs, in_=ssum)
        lgsb = const.tile([128, NB, E], F32, tag="lgsb")
        nc.vector.tensor_copy(out=lgsb, in_=lg_ps)
        mx = const.tile([128, NB, 8], F32, tag="mx")
        mi = const.tile([128, NB, 8], U32, tag="mi")
        for nb in range(NB):
            nc.vector.max_with_indices(mx[:, nb, :], mi[:, nb, :], lgsb[:, nb, :])
        dump("d_lgsb", lgsb)
        dump("d_mx", mx)
        dump("d_mi", mi)
        dump("d_el", el)
        dump("d_ssum", ssum)
        wiv = const.tile([128, NB, 4], F32, tag="wiv")
        ev = const.tile([128, NB, 2], F32, tag="ev")
        nc.scalar.activation(out=ev, in_=mx[:, :, 0:2], func=EXP)
        for nb in range(NB):
            nc.vector.tensor_scalar_mul(
                wiv[:, nb, 0:2], ev[:, nb, :], rs[:, nb:nb + 1]
            )
        nc.vector.tensor_copy(out=wiv[:, :, 2:4], in_=mi[:, :, 0:2])

        # ---- transpose w0, w1, i0, i1 each to its own [1, NT] row tile ----
        rows = []  # w0T, w1T, i0T, i1T (bf16: w is bf16 downstream anyway,
        # idx values 0..7 are exact)
        for j in range(4):
            rt = const.tile([1, NT], BF16, tag=f"rowT{j}")
            for nb in range(NB):
                tps = pC.tile([1, 128], F32, tag="c")
                nc.tensor.transpose(tps, wiv[:, nb, j:j + 1], ident)
                nc.vector.tensor_copy(
                    out=rt[:, nb * 128:(nb + 1) * 128], in_=tps
                )
            rows.append(rt)
            dump(f"d_row{j}", rt)
        dump("d_wiv", wiv)

        z2w0 = const.tile([ER, NT], BF16, tag="z2w0")
        z2w1 = const.tile([ER, NT], BF16, tag="z2w1")
        swT = const.tile([1, NT], BF16, tag="swT")
        nc.vector.tensor_tensor(out=swT, in0=rows[0], in1=rows[1], op=ADD)

        # ---- expert masks ohT_k [ER, NT] and weight rows wrep_k [128, NT] ----
        oh = []
        wrep = []
        for k in range(2):
            rep_ps = pC.tile([ER, NT], F32, tag="c")
            nc.tensor.matmul(
                rep_ps, onesb[0:1, 0:ER], rows[2 + k], start=True, stop=True
            )
            ohk = const.tile([ER, NT], BF16, tag=f"oh{k}")
            nc.vector.tensor_scalar(
                out=ohk, in0=rep_ps, scalar1=eoer, scalar2=None, op0=ISEQ
            )
            oh.append(ohk)
            dump(f"d_oh{k}", ohk)
            wr_ps = pC.tile([128, NT], F32, tag="c")
            nc.tensor.matmul(wr_ps, onesb, rows[k], start=True, stop=True)
            wrk = const.tile([128, NT], BF16, tag=f"wrep{k}")
            nc.vector.tensor_copy(out=wrk, in_=wr_ps)
            wrep.append(wrk)
            dump(f"d_wrep{k}", wrk)

        # ---- u1 = A1_flat @ xT, masked -> z1_k ----
        u1_ps = pU.tile([ER, NT], F32, tag="u0")
        for dc in range(DC):
            nc.tensor.matmul(
                u1_ps, a1t[:, dc, :], xbf[:, dc, :],
                start=(dc == 0), stop=(dc == DC - 1),
            )
        if dbg:
            du1 = const.tile([ER, NT], F32, tag="du1")
            nc.vector.tensor_copy(out=du1, in_=u1_ps)
            dump("d_u1", du1)
        z1_0 = const.tile([ER, NT], BF16, tag="z1_0")
        nc.vector.tensor_tensor(out=z1_0, in0=u1_ps, in1=oh[0], op=MULT)
        dump("d_z10", z1_0)
        # delta mask: z1d = u1 * (oh1 - oh0); base+c0+B1s@z1d == base+c1
        ohd = const.tile([ER, NT], BF16, tag="ohd")
        nc.vector.tensor_tensor(
            out=ohd, in0=oh[1], in1=oh[0], op=mybir.AluOpType.subtract
        )
        z1d = const.tile([ER, NT], BF16, tag="z1d")
        nc.vector.tensor_tensor(out=z1d, in0=u1_ps, in1=ohd, op=MULT)

        # ---- fc1 loop over f-block PAIRS ----
        # Per pair: 32 base matmuls back-to-back, then ONE cluster of small
        # (partial-array) matmuls: c0 for this pair, the z1d expert-delta for
        # the previous pair, and u2 for the pair before that. Clustering the
        # row_grp/col_grp matmuls halves full/partial LDWEIGHTS transitions.
        u20_ps = pU.tile([ER, NT], F32, tag="u0")
        u21_ps = pU.tile([ER, NT], F32, tag="u1")
        pend_c1d = []  # [(fb, fbs, bank, h0)] awaiting the delta half
        pend_u2 = []   # [(fb, h0, h1)] awaiting u2/hsum

        def flush_u2():
            for fbq, h0q, h1q in pend_u2:
                nc.tensor.matmul(
                    u20_ps, a2t[:, fbq, :], h0q,
                    start=(fbq == 0), stop=(fbq == FB - 1),
                )
                nc.tensor.matmul(
                    u21_ps, a2t[:, fbq, :], h1q,
                    start=(fbq == 0), stop=(fbq == FB - 1),
                )
            for fbq, h0q, h1q in pend_u2:
                t0 = work.tile([128, NT], BF16, tag="t0")
                nc.vector.tensor_tensor(out=t0, in0=h0q, in1=wrep[0], op=MULT)
                t1 = work.tile([128, NT], BF16, tag="t1")
                nc.vector.tensor_tensor(out=t1, in0=h1q, in1=wrep[1], op=MULT)
                nc.vector.tensor_tensor(
                    out=hsum[:, fbq, :], in0=t0, in1=t1, op=ADD
                )
            pend_u2.clear()

        def flush_c1d():
            done = []
            for fbq, fbsq, bank, h0q in pend_c1d:
                nc.tensor.matmul(
                    bank, b1s[:, fbsq], z1d, start=False, stop=True,
                    skip_group_check=True,
                )
                h1q = hwork.tile([128, NT], BF16, tag="h1", bufs=4)
                nc.scalar.activation(
                    out=h1q, in_=bank, func=GELU, bias=b1r[:, fbq:fbq + 1]
                )
                done.append((fbq, h0q, h1q))
            pend_c1d.clear()
            return done

        for p in range(FB // 2):
            newc1d = []
            for fb in (2 * p, 2 * p + 1):
                fbs = slice(fb * 128, (fb + 1) * 128)
                if fb in w1sbs:
                    w1sb = w1sbs.pop(fb)
                else:
                    w1sb = w1p.tile([128, DC, 128], BF16, tag="w1")
                    nc.sync.dma_start(out=w1sb, in_=a["w1t"][fb])
                base_ps = pA.tile([128, NT], F32, tag="a")
                for dc in range(DC):
                    nc.tensor.matmul(
                        base_ps, w1sb[:, dc, :], xbf[:, dc, :],
                        start=(dc == 0), stop=False,
                    )
                newc1d.append((fb, fbs, base_ps))
            # small-matmul cluster: c0 for this pair first (gates gelu-h0)
            augmented = []
            for fb, fbs, bank in newc1d:
                nc.tensor.matmul(
                    bank, b1s[:, fbs], z1_0, start=False, stop=True,
                    skip_group_check=True,
                )
                h0 = hwork.tile([128, NT], BF16, tag="h0", bufs=6)
                nc.scalar.activation(
                    out=h0, in_=bank, func=GELU, bias=b1r[:, fb:fb + 1]
                )
                augmented.append((fb, fbs, bank, h0))
            done = flush_c1d()
            flush_u2()
            pend_u2.extend(done)
            pend_c1d.extend(augmented)
        pend_u2.extend(flush_c1d())
        flush_u2()
        dump("d_hsum", hsum)

        # ---- z2w_k = u2_k * oh_k * w_k ----
        for k, u2_ps in ((0, u20_ps), (1, u21_ps)):
            if dbg:
                du2 = const.tile([ER, NT], F32, tag=f"du2{k}")
                nc.vector.tensor_copy(out=du2, in_=u2_ps)
                dump(f"d_u2{k}", du2)
            tz = work.tile([ER, NT], F32, tag="tz")
            nc.vector.tensor_tensor(out=tz, in0=u2_ps, in1=oh[k], op=MULT)
            dst = z2w0 if k == 0 else z2w1
            nc.vector.tensor_tensor(out=dst, in0=tz, in1=wrep[k][0:ER, :], op=MULT)
            dump(f"d_z2w{k}", dst)

        # ---- fc2 loop over d-block PAIRS (small matmuls clustered) ----
        FH = FB // 2  # load W2 per d-block in two halves
        for dp in range(DB // 2):
            items = []
            for db in (2 * dp, 2 * dp + 1):
                dbs = slice(db * 128, (db + 1) * 128)
                w2h = []
                for h in range(2):
                    t = w2p.tile([128, FH, 128], BF16, tag="w2")
                    nc.sync.dma_start(
                        out=t, in_=a["w2t"][db][:, h * FH:(h + 1) * FH, :]
                    )
                    w2h.append(t)
                y_ps = pA.tile([128, NT], F32, tag="a")
                for fc in range(FB):
                    nc.tensor.matmul(
                        y_ps, w2h[fc // FH][:, fc % FH, :], hsum[:, fc, :],
                        start=(fc == 0), stop=False,
                    )
                items.append((dbs, y_ps))
            for dbs, y_ps in items:
                nc.tensor.matmul(
                    y_ps, b2s[:, dbs], z2w0, start=False, stop=False
                )
                nc.tensor.matmul(
                    y_ps, b2s[:, dbs], z2w1, start=False, stop=False
                )
                nc.tensor.matmul(y_ps, b2t[:, dbs], swT, start=False, stop=True)
            for dbs, y_ps in items:
                osb = work.tile([128, NT], F32, tag="osb")
                nc.vector.tensor_copy(out=osb, in_=y_ps)
                nc.sync.dma_start(out=a["rest"][dbs, :], in_=osb)


def _build(cfg):
    nc = bacc.Bacc("TRN2", target_bir_lowering=False, debug=False)
    D, F, E, R, NT = cfg["D"], cfg["F"], cfg["E"], cfg["R"], cfg["NT"]
    DC, FB = D // 128, F // 128
    ER = E * R

    def din(name, shape, dt):
        return nc.dram_tensor(name, shape, dt, kind="ExternalInput").ap()

    a = {
        "xt": din("xt", [128, DC, NT], F32),
        "xb": din("xb", [128, DC, NT], BF16),
        "w1t": din("w1t", [FB, 128, DC, 128], BF16),
        "w2t": din("w2t", [DC, 128, FB, 128], BF16),
        "wrt": din("wrt", [128, DC, E], F32),
        "brow": din("brow", [1, E], F32),
        "a1t": din("a1t", [128, DC, ER], BF16),
        "a2t": din("a2t", [128, FB, ER], BF16),
        "b1s": din("b1s", [ER, F], BF16),
        "b2s": din("b2s", [ER, D], BF16),
        "b2t": din("b2t", [1, D], BF16),
        "b1r": din("b1r", [128, FB], F32),
        "ident": din("ident", [128, 128], F32),
        "ones": din("ones", [1, 128], F32),
        "onesb": din("onesb", [1, 128], BF16),
        "eoer": din("eoer", [ER, 1], F32),
        "rest": nc.dram_tensor("rest", [D, NT], F32, kind="ExternalOutput").ap(),
    }
    if cfg.get("debug", False):
        def dout(name, shape, dt):
            a[name] = nc.dram_tensor(name, shape, dt, kind="ExternalOutput").ap()

        NB = NT // 128
        for j in range(4):
            dout(f"d_row{j}", [1, NT], F32)
        dout("d_lgsb", [128, NB, E], F32)
        dout("d_mx", [128, NB, 8], F32)
        dout("d_mi", [128, NB, 8], U32)
        dout("d_el", [128, NB, E], F32)
        dout("d_ssum", [128, NB], F32)
        dout("d_wiv", [128, NB, 4], F32)
        for k in range(2):
            dout(f"d_oh{k}", [ER, NT], BF16)
            dout(f"d_wrep{k}", [128, NT], F32)
            dout(f"d_z1{k}", [ER, NT], BF16)
            dout(f"d_u2{k}", [ER, NT], F32)
            dout(f"d_z2w{k}", [ER, NT], BF16)
        dout("d_u1", [ER, NT], F32)
        dout("d_base0", [128, NT], F32)
        dout("d_h0", [128, NT], BF16)
        dout("d_h1", [128, NT], BF16)
        dout("d_hsum", [128, FB, NT], BF16)
    with tile.TileContext(nc) as tc:
        _emit(nc, tc, a, cfg)
    nc.compile()
    return nc


def _host_prep(hidden_states, W_r, b_r, W1, b1, W2, b2, A1, B1, A2, B2):
    """Layout prep on host. Returns (in_maps, cfg)."""
    x = np.ascontiguousarray(np.asarray(hidden_states, dtype=np.float32))
    B_, S_, D = x.shape
    N = B_ * S_
    NT = N // N_CORES
    F = np.asarray(W1).shape[0]
    E, R = np.asarray(A1).shape[0], np.asarray(A1).shape[1]
    ER = E * R
    DC, FB = D // 128, F // 128
    SCALING = 2.0

    W_r = np.asarray(W_r, dtype=np.float32)
    W1 = np.asarray(W1, dtype=np.float32)
    W2 = np.asarray(W2, dtype=np.float32)
    A1 = np.asarray(A1, dtype=np.float32)
    B1 = np.asarray(B1, dtype=np.float32)
    A2 = np.asarray(A2, dtype=np.float32)
    B2 = np.asarray(B2, dtype=np.float32)
    b_r = np.asarray(b_r, dtype=np.float32)
    b1 = np.asarray(b1, dtype=np.float32)
    b2 = np.asarray(b2, dtype=np.float32)

    bf = ml_dtypes.bfloat16
    w1t = np.ascontiguousarray(
        W1.reshape(FB, 128, DC, 128).transpose(0, 3, 2, 1)
    ).astype(bf)
    w2t = np.ascontiguousarray(
        W2.reshape(DC, 128, FB, 128).transpose(0, 3, 2, 1)
    ).astype(bf)
    wrt = np.ascontiguousarray(W_r.T.reshape(DC, 128, E).transpose(1, 0, 2))
    a1t = np.ascontiguousarray(
        A1.reshape(ER, D).T.reshape(DC, 128, ER).transpose(1, 0, 2)
    ).astype(bf)
    a2t = np.ascontiguousarray(
        A2.reshape(ER, F).T.reshape(FB, 128, ER).transpose(1, 0, 2)
    ).astype(bf)
    b1s = (B1.transpose(0, 2, 1).reshape(ER, F) * SCALING).astype(bf)
    b2s = (B2.transpose(0, 2, 1).reshape(ER, D) * SCALING).astype(bf)
    b2t = b2[None, :].astype(bf)
    b1r = np.ascontiguousarray(b1.reshape(FB, 128).T)
    shared = {
        "w1t": w1t,
        "w2t": w2t,
        "wrt": wrt,
        "brow": np.ascontiguousarray(b_r[None, :]),
        "a1t": a1t,
        "a2t": a2t,
        "b1s": b1s,
        "b2s": b2s,
        "b2t": b2t,
        "b1r": b1r,
        "ident": np.eye(128, dtype=np.float32),
        "ones": np.ones((1, 128), dtype=np.float32),
        "onesb": np.ones((1, 128), dtype=bf),
        "eoer": (np.arange(ER, dtype=np.float32) // R).reshape(ER, 1),
    }
    xf = x.reshape(N, D)
    in_maps = []
    for c in range(N_CORES):
        xc = xf[c * NT:(c + 1) * NT]
        xt = np.ascontiguousarray(xc.reshape(NT, DC, 128).transpose(2, 1, 0))
        in_maps.append({"xt": xt, "xb": xt.astype(bf), **shared})
    cfg = {"D": D, "F": F, "E": E, "R": R, "NT": NT}
    return in_maps, cfg, (B_, S_, N)


_nc_cache = {}


def _run(inputs, trace=False, trace_cores=None, debug=False):
    in_maps, cfg, (B_, S_, N) = _host_prep(**inputs)
    if debug:
        cfg["debug"] = True
    key = tuple(sorted(cfg.items()))
    if key not in _nc_cache:
        _nc_cache[key] = _build(cfg)
    nc = _nc_cache[key]
    res = bass_utils.run_bass_kernel_spmd(
        nc,
        in_maps,
        core_ids=list(range(N_CORES)),
        trace=trace,
        trace_cores=trace_cores,
    )
    D, NT = cfg["D"], cfg["NT"]
    out = np.empty((N, D), dtype=np.float32)
    for c in range(N_CORES):
        out[c * NT:(c + 1) * NT] = res.results[c]["rest"].T
    return out.reshape(B_, S_, D), res


def kernel(**inputs):
    out, _ = _run(inputs)
    return out


# revision 82
# speedup vs baseline: 1.0106x; 1.0013x over previous
"""Trainium2 Bass kernel for nn_Experts (moe_routing, LoRA-per-expert MLP).

Self-contained: kernel(**inputs) -> np.ndarray [B, S, D] float32.

Strategy: data-parallel over tokens across 8 NeuronCores (512 tokens/core),
base weights replicated. Per core, everything is computed in transposed
activation layout ([feature, token]) so all GEMM operands are natural-layout
SBUF tiles with the contraction on the partition axis:

  router (fp32):  logits[n,e] = x@W_r.T + b_r; softmax via ACT-exp;
                  top-2 via DVE max_with_indices (descending, = top_k order)
  fc1 (bf16):     base1T = W1 @ xT accumulated in PSUM; per-k LoRA correction
                  (z1_k @ B1s) accumulated into the same bank; gelu_tanh+b1
                  fused on ACT
  combine:        hsumT = w0*h0T + w1*h1T  (fc2 is linear in h, so one
                  shared fc2 GEMM instead of two)
  fc2 (bf16):     yT = W2 @ hsumT + sum_k (w_k*z2_k) @ B2s + b2 ⊗ (w0+w1),
                  the bias via a rank-1 matmul accumulated into the same bank

Host side only reshapes/transposes/casts (layout prep), no model math.
"""

import numpy as np
import ml_dtypes

import concourse.bacc as bacc
import concourse.tile as tile
from concourse import mybir
from concourse import bass_utils

BF16 = mybir.dt.bfloat16
F32 = mybir.dt.float32
U32 = mybir.dt.uint32

GELU = mybir.ActivationFunctionType.Gelu_apprx_tanh
EXP = mybir.ActivationFunctionType.Exp
ADD = mybir.AluOpType.add
MULT = mybir.AluOpType.mult
ISEQ = mybir.AluOpType.is_equal

N_CORES = 8


def _emit(nc, tc, a, cfg):
    """Emit the per-core program. `a` maps name -> bass.AP (dram)."""
    import contextlib

    D, F, E, R, NT = cfg["D"], cfg["F"], cfg["E"], cfg["R"], cfg["NT"]
    DC, FB = D // 128, F // 128  # d-chunks, f-blocks
    DB = D // 128                # output d-blocks
    NB = NT // 128               # router n-blocks
    ER = E * R

    dbg = cfg.get("debug", False)

    def dump(name, tile_ap):
        if dbg and name in a:
            nc.sync.dma_start(out=a[name], in_=tile_ap)

    with contextlib.ExitStack() as ctx:
        ec = ctx.enter_context
        const = ec(tc.tile_pool(name="const", bufs=1))
        xf32p = ec(tc.tile_pool(name="xf32p", bufs=3))
        w1p = ec(tc.tile_pool(name="w1p", bufs=3))
        w2p = ec(tc.tile_pool(name="w2p", bufs=2))
        work = ec(tc.tile_pool(name="work", bufs=2))
        hwork = ec(tc.tile_pool(name="hwork", bufs=4))
        pA = ec(tc.tile_pool(name="pA", bufs=4, space="PSUM"))
        pC = ec(tc.tile_pool(name="pC", bufs=2, space="PSUM"))
        pU = ec(tc.tile_pool(name="pU", bufs=1, space="PSUM"))

        # ---- DMA staging: router weight + x tiles first (router starts
        # ~immediately; bf16 x is derived on-chip per-tile), then W1 blocks.
        wrt = const.tile([128, DC, E], F32, tag="wrt")
        nc.sync.dma_start(out=wrt, in_=a["wrt"])
        xfs = []
        for dc in range(DC):
            xf = xf32p.tile([128, NT], F32, tag="xf", name=f"xf{dc}", bufs=5)
            nc.sync.dma_start(out=xf, in_=a["xt"][:, dc, :])
            xfs.append(xf)
        xbf = const.tile([128, DC, NT], BF16, tag="xbf")
        W1_PRE = 3
        w1sbs = {}
        for fb in range(W1_PRE):
            w1sbs[fb] = w1p.tile(
                [128, DC, 128], BF16, tag="w1", name=f"w1pre{fb}"
            )
            nc.sync.dma_start(out=w1sbs[fb], in_=a["w1t"][fb])

        hsum = const.tile([128, FB, NT], BF16, tag="hsum")

        # ---- router logits (fp32) + on-chip bf16 cast of x ----
        lg_ps = pC.tile([128, NB, E], F32, tag="c")
        for dc in range(DC):
            xf = xfs[dc]
            nc.scalar.copy(out=xbf[:, dc, :], in_=xf)
            for nb in range(NB):
                # start=True clears the WHOLE psum bank, so only the very
                # first matmul into this bank may set it.
                nc.tensor.matmul(
                    lg_ps[:, nb, :],
                    xf[:, nb * 128:(nb + 1) * 128],
                    wrt[:, dc, :],
                    start=(dc == 0 and nb == 0),
                    stop=False,
                )

        # ---- remaining resident constants (needed later than the router) ----
        ident = const.tile([128, 128], F32, tag="ident")
        nc.sync.dma_start(out=ident, in_=a["ident"])
        ones = const.tile([1, 128], F32, tag="ones")
        nc.sync.dma_start(out=ones, in_=a["ones"])
        onesb = const.tile([1, 128], BF16, tag="onesb")
        nc.sync.dma_start(out=onesb, in_=a["onesb"])
        eoer = const.tile([ER, 1], F32, tag="eoer")
        nc.sync.dma_start(out=eoer, in_=a["eoer"])
        b1r = const.tile([128, FB], F32, tag="b1r")
        nc.sync.dma_start(out=b1r, in_=a["b1r"])
        brow = const.tile([1, E], F32, tag="brow")
        nc.sync.dma_start(out=brow, in_=a["brow"])
        a1t = const.tile([128, DC, ER], BF16, tag="a1t")
        nc.sync.dma_start(out=a1t, in_=a["a1t"])
        a2t = const.tile([128, FB, ER], BF16, tag="a2t")
        nc.sync.dma_start(out=a2t, in_=a["a2t"])
        b1s = const.tile([ER, F], BF16, tag="b1s")
        nc.sync.dma_start(out=b1s, in_=a["b1s"])
        b2sc = const.tile([2 * ER, D], BF16, tag="b2sc")
        nc.sync.dma_start(out=b2sc, in_=a["b2sc"])
        b2t = const.tile([1, D], BF16, tag="b2t")
        nc.sync.dma_start(out=b2t, in_=a["b2t"])
        for nb in range(NB):  # + ones ⊗ b_r
            nc.tensor.matmul(
                lg_ps[:, nb, :], ones, brow, start=False, stop=True
            )

        # ---- softmax denominators + top-2 ----
        el = const.tile([128, NB, E], F32, tag="el")
        nc.scalar.activation(out=el, in_=lg_ps, func=EXP)
        ssum = const.tile([128, NB], F32, tag="ssum")
        nc.vector.reduce_sum(out=ssum, in_=el, axis=mybir.AxisListType.X)
        rs = const.tile([128, NB], F32, tag="rs")
        nc.vector.reciprocal(out=rs, in_=ssum)
        lgsb = const.tile([128, NB, E], F32, tag="lgsb")
        nc.vector.tensor_copy(out=lgsb, in_=lg_ps)
        mx = const.tile([128, NB, 8], F32, tag="mx")
        mi = const.tile([128, NB, 8], U32, tag="mi")
        for nb in range(NB):
            nc.vector.max_with_indices(mx[:, nb, :], mi[:, nb, :], lgsb[:, nb, :])
        dump("d_lgsb", lgsb)
        dump("d_mx", mx)
        dump("d_mi", mi)
        dump("d_el", el)
        dump("d_ssum", ssum)
        wiv = const.tile([128, NB, 4], F32, tag="wiv")
        ev = const.tile([128, NB, 2], F32, tag="ev")
        nc.scalar.activation(out=ev, in_=mx[:, :, 0:2], func=EXP)
        for nb in range(NB):
            nc.vector.tensor_scalar_mul(
                wiv[:, nb, 0:2], ev[:, nb, :], rs[:, nb:nb + 1]
            )
        nc.vector.tensor_copy(out=wiv[:, :, 2:4], in_=mi[:, :, 0:2])

        # ---- transpose w0, w1, i0, i1 each to its own [1, NT] row tile ----
        rows = []  # w0T, w1T, i0T, i1T (bf16: w is bf16 downstream anyway,
        # idx values 0..7 are exact)
        for j in range(4):
            rt = const.tile([1, NT], BF16, tag=f"rowT{j}")
            for nb in range(NB):
                tps = pC.tile([1, 128], F32, tag="c")
                nc.tensor.transpose(tps, wiv[:, nb, j:j + 1], ident)
                nc.vector.tensor_copy(
                    out=rt[:, nb * 128:(nb + 1) * 128], in_=tps
                )
            rows.append(rt)
            dump(f"d_row{j}", rt)
        dump("d_wiv", wiv)

        # z2w for both experts stacked on the contraction axis -> fc2 LoRA is
        # ONE full-array matmul per d-block (b2sc = [B2s; B2s]).
        z2wcat = const.tile([2 * ER, NT], BF16, tag="z2wcat")
        z2w1t = const.tile([ER, NT], BF16, tag="z2w1t")
        swT = const.tile([1, NT], BF16, tag="swT")
        nc.vector.tensor_tensor(out=swT, in0=rows[0], in1=rows[1], op=ADD)

        # ---- expert masks ohT_k [ER, NT] and weight rows wrep_k [128, NT] ----
        oh = []
        wrep = []
        for k in range(2):
            rep_ps = pC.tile([ER, NT], F32, tag="c")
            nc.tensor.matmul(
                rep_ps, onesb[0:1, 0:ER], rows[2 + k], start=True, stop=True
            )
            ohk = const.tile([ER, NT], BF16, tag=f"oh{k}")
            nc.vector.tensor_scalar(
                out=ohk, in0=rep_ps, scalar1=eoer, scalar2=None, op0=ISEQ
            )
            oh.append(ohk)
            dump(f"d_oh{k}", ohk)
            wr_ps = pC.tile([128, NT], F32, tag="c")
            nc.tensor.matmul(wr_ps, onesb, rows[k], start=True, stop=True)
            wrk = const.tile([128, NT], BF16, tag=f"wrep{k}")
            nc.vector.tensor_copy(out=wrk, in_=wr_ps)
            wrep.append(wrk)
            dump(f"d_wrep{k}", wrk)

        # ---- u1 = A1_flat @ xT, masked -> z1_k ----
        u1_ps = pU.tile([ER, NT], F32, tag="u0")
        for dc in range(DC):
            nc.tensor.matmul(
                u1_ps, a1t[:, dc, :], xbf[:, dc, :],
                start=(dc == 0), stop=(dc == DC - 1),
            )
        if dbg:
            du1 = const.tile([ER, NT], F32, tag="du1")
            nc.vector.tensor_copy(out=du1, in_=u1_ps)
            dump("d_u1", du1)
        z1_0 = const.tile([ER, NT], BF16, tag="z1_0")
        nc.vector.tensor_tensor(out=z1_0, in0=u1_ps, in1=oh[0], op=MULT)
        dump("d_z10", z1_0)
        # delta mask: z1d = u1 * (oh1 - oh0); base+c0+B1s@z1d == base+c1
        ohd = const.tile([ER, NT], BF16, tag="ohd")
        nc.vector.tensor_tensor(
            out=ohd, in0=oh[1], in1=oh[0], op=mybir.AluOpType.subtract
        )
        z1d = const.tile([ER, NT], BF16, tag="z1d")
        nc.vector.tensor_tensor(out=z1d, in0=u1_ps, in1=ohd, op=MULT)

        # ---- fc1 loop over f-block PAIRS ----
        # Per pair: 32 base matmuls back-to-back, then ONE cluster of small
        # (partial-array) matmuls: c0 for this pair, the z1d expert-delta for
        # the previous pair, and u2 for the pair before that. Clustering the
        # row_grp/col_grp matmuls halves full/partial LDWEIGHTS transitions.
        u20_ps = pU.tile([ER, NT], F32, tag="u0")
        u21_ps = pU.tile([ER, NT], F32, tag="u1")
        pend_c1d = []  # [(fb, fbs, bank, h0)] awaiting the delta half
        pend_u2 = []   # [(fb, h0, h1)] awaiting u2/hsum

        def flush_u2():
            for fbq, h0q, h1q in pend_u2:
                nc.tensor.matmul(
                    u20_ps, a2t[:, fbq, :], h0q,
                    start=(fbq == 0), stop=(fbq == FB - 1),
                )
                nc.tensor.matmul(
                    u21_ps, a2t[:, fbq, :], h1q,
                    start=(fbq == 0), stop=(fbq == FB - 1),
                )
            for fbq, h0q, h1q in pend_u2:
                t0 = work.tile([128, NT], BF16, tag="t0")
                nc.vector.tensor_tensor(out=t0, in0=h0q, in1=wrep[0], op=MULT)
                t1 = work.tile([128, NT], BF16, tag="t1")
                nc.vector.tensor_tensor(out=t1, in0=h1q, in1=wrep[1], op=MULT)
                nc.vector.tensor_tensor(
                    out=hsum[:, fbq, :], in0=t0, in1=t1, op=ADD
                )
            pend_u2.clear()

        def flush_c1d():
            done = []
            for fbq, fbsq, bank, h0q in pend_c1d:
                nc.tensor.matmul(
                    bank, b1s[:, fbsq], z1d, start=False, stop=True,
                    skip_group_check=True,
                )
                h1q = hwork.tile([128, NT], BF16, tag="h1", bufs=4)
                nc.scalar.activation(
                    out=h1q, in_=bank, func=GELU, bias=b1r[:, fbq:fbq + 1]
                )
                done.append((fbq, h0q, h1q))
            pend_c1d.clear()
            return done

        for p in range(FB // 2):
            newc1d = []
            for fb in (2 * p, 2 * p + 1):
                fbs = slice(fb * 128, (fb + 1) * 128)
                if fb in w1sbs:
                    w1sb = w1sbs.pop(fb)
                else:
                    w1sb = w1p.tile([128, DC, 128], BF16, tag="w1")
                    nc.sync.dma_start(out=w1sb, in_=a["w1t"][fb])
                base_ps = pA.tile([128, NT], F32, tag="a")
                for dc in range(DC):
                    nc.tensor.matmul(
                        base_ps, w1sb[:, dc, :], xbf[:, dc, :],
                        start=(dc == 0), stop=False,
                    )
                newc1d.append((fb, fbs, base_ps))
            # small-matmul cluster: c0 for this pair first (gates gelu-h0)
            augmented = []
            for fb, fbs, bank in newc1d:
                nc.tensor.matmul(
                    bank, b1s[:, fbs], z1_0, start=False, stop=True,
                    skip_group_check=True,
                )
                h0 = hwork.tile([128, NT], BF16, tag="h0", bufs=6)
                nc.scalar.activation(
                    out=h0, in_=bank, func=GELU, bias=b1r[:, fb:fb + 1]
                )
                augmented.append((fb, fbs, bank, h0))
            done = flush_c1d()
            flush_u2()
            pend_u2.extend(done)
            pend_c1d.extend(augmented)
        pend_u2.extend(flush_c1d())
        flush_u2()
        dump("d_hsum", hsum)

        # ---- z2w_k = u2_k * oh_k * w_k ----
        for k, u2_ps in ((0, u20_ps), (1, u21_ps)):
            if dbg:
                du2 = const.tile([ER, NT], F32, tag=f"du2{k}")
                nc.vector.tensor_copy(out=du2, in_=u2_ps)
                dump(f"d_u2{k}", du2)
            tz = work.tile([ER, NT], F32, tag="tz")
            nc.vector.tensor_tensor(out=tz, in0=u2_ps, in1=oh[k], op=MULT)
            dst = z2wcat[0:ER, :] if k == 0 else z2w1t
            nc.vector.tensor_tensor(out=dst, in0=tz, in1=wrep[k][0:ER, :], op=MULT)
            dump(f"d_z2w{k}", dst)
        # partition-offset move into rows ER..2ER via SBUF->SBUF DMA
        nc.sync.dma_start(out=z2wcat[ER:2 * ER, :], in_=z2w1t)

        # ---- fc2 loop over d-block PAIRS (small matmuls clustered) ----
        FH = FB // 2  # load W2 per d-block in two halves
        for dp in range(DB // 2):
            items = []
            for db in (2 * dp, 2 * dp + 1):
                dbs = slice(db * 128, (db + 1) * 128)
                w2h = []
                for h in range(2):
                    t = w2p.tile([128, FH, 128], BF16, tag="w2")
                    nc.sync.dma_start(
                        out=t, in_=a["w2t"][db][:, h * FH:(h + 1) * FH, :]
                    )
                    w2h.append(t)
                y_ps = pA.tile([128, NT], F32, tag="a")
                for fc in range(FB):
                    nc.tensor.matmul(
                        y_ps, w2h[fc // FH][:, fc % FH, :], hsum[:, fc, :],
                        start=(fc == 0), stop=False,
                    )
                items.append((dbs, y_ps))
            for dbs, y_ps in items:
                nc.tensor.matmul(
                    y_ps, b2sc[:, dbs], z2wcat, start=False, stop=False
                )
                nc.tensor.matmul(y_ps, b2t[:, dbs], swT, start=False, stop=True)
            for dbs, y_ps in items:
                osb = work.tile([128, NT], F32, tag="osb")
                nc.vector.tensor_copy(out=osb, in_=y_ps)
                nc.sync.dma_start(out=a["rest"][dbs, :], in_=osb)


def _build(cfg):
    nc = bacc.Bacc("TRN2", target_bir_lowering=False, debug=False)
    D, F, E, R, NT = cfg["D"], cfg["F"], cfg["E"], cfg["R"], cfg["NT"]
    DC, FB = D // 128, F // 128
    ER = E * R

    def din(name, shape, dt):
        return nc.dram_tensor(name, shape, dt, kind="ExternalInput").ap()

    a = {
        "xt": din("xt", [128, DC, NT], F32),
        "xb": din("xb", [128, DC, NT], BF16),
        "w1t": din("w1t", [FB, 128, DC, 128], BF16),
        "w2t": din("w2t", [DC, 128, FB, 128], BF16),
        "wrt": din("wrt", [128, DC, E], F32),
        "brow": din("brow", [1, E], F32),
        "a1t": din("a1t", [128, DC, ER], BF16),
        "a2t": din("a2t", [128, FB, ER], BF16),
        "b1s": din("b1s", [ER, F], BF16),
        "b2sc": din("b2sc", [2 * ER, D], BF16),
        "b2t": din("b2t", [1, D], BF16),
        "b1r": din("b1r", [128, FB], F32),
        "ident": din("ident", [128, 128], F32),
        "ones": din("ones", [1, 128], F32),
        "onesb": din("onesb", [1, 128], BF16),
        "eoer": din("eoer", [ER, 1], F32),
        "rest": nc.dram_tensor("rest", [D, NT], F32, kind="ExternalOutput").ap(),
    }
    if cfg.get("debug", False):
        def dout(name, shape, dt):
            a[name] = nc.dram_tensor(name, shape, dt, kind="ExternalOutput").ap()

        NB = NT // 128
        for j in range(4):
            dout(f"d_row{j}", [1, NT], F32)
        dout("d_lgsb", [128, NB, E], F32)
        dout("d_mx", [128, NB, 8], F32)
        dout("d_mi", [128, NB, 8], U32)
        dout("d_el", [128, NB, E], F32)
        dout("d_ssum", [128, NB], F32)
        dout("d_wiv", [128, NB, 4], F32)
        for k in range(2):
            dout(f"d_oh{k}", [ER, NT], BF16)
            dout(f"d_wrep{k}", [128, NT], F32)
            dout(f"d_z1{k}", [ER, NT], BF16)
            dout(f"d_u2{k}", [ER, NT], F32)
            dout(f"d_z2w{k}", [ER, NT], BF16)
        dout("d_u1", [ER, NT], F32)
        dout("d_base0", [128, NT], F32)
        dout("d_h0", [128, NT], BF16)
        dout("d_h1", [128, NT], BF16)
        dout("d_hsum", [128, FB, NT], BF16)
    with tile.TileContext(nc) as tc:
        _emit(nc, tc, a, cfg)
    nc.compile()
    return nc


def _host_prep(hidden_states, W_r, b_r, W1, b1, W2, b2, A1, B1, A2, B2):
    """Layout prep on host. Returns (in_maps, cfg)."""
    x = np.ascontiguousarray(np.asarray(hidden_states, dtype=np.float32))
    B_, S_, D = x.shape
    N = B_ * S_
    NT = N // N_CORES
    F = np.asarray(W1).shape[0]
    E, R = np.asarray(A1).shape[0], np.asarray(A1).shape[1]
    ER = E * R
    DC, FB = D // 128, F // 128
    SCALING = 2.0

    W_r = np.asarray(W_r, dtype=np.float32)
    W1 = np.asarray(W1, dtype=np.float32)
    W2 = np.asarray(W2, dtype=np.float32)
    A1 = np.asarray(A1, dtype=np.float32)
    B1 = np.asarray(B1, dtype=np.float32)
    A2 = np.asarray(A2, dtype=np.float32)
    B2 = np.asarray(B2, dtype=np.float32)
    b_r = np.asarray(b_r, dtype=np.float32)
    b1 = np.asarray(b1, dtype=np.float32)
    b2 = np.asarray(b2, dtype=np.float32)

    bf = ml_dtypes.bfloat16
    w1t = np.ascontiguousarray(
        W1.reshape(FB, 128, DC, 128).transpose(0, 3, 2, 1)
    ).astype(bf)
    w2t = np.ascontiguousarray(
        W2.reshape(DC, 128, FB, 128).transpose(0, 3, 2, 1)
    ).astype(bf)
    wrt = np.ascontiguousarray(W_r.T.reshape(DC, 128, E).transpose(1, 0, 2))
    a1t = np.ascontiguousarray(
        A1.reshape(ER, D).T.reshape(DC, 128, ER).transpose(1, 0, 2)
    ).astype(bf)
    a2t = np.ascontiguousarray(
        A2.reshape(ER, F).T.reshape(FB, 128, ER).transpose(1, 0, 2)
    ).astype(bf)
    b1s = (B1.transpose(0, 2, 1).reshape(ER, F) * SCALING).astype(bf)
    b2s1 = (B2.transpose(0, 2, 1).reshape(ER, D) * SCALING).astype(bf)
    b2sc = np.concatenate([b2s1, b2s1], axis=0)
    b2t = b2[None, :].astype(bf)
    b1r = np.ascontiguousarray(b1.reshape(FB, 128).T)
    shared = {
        "w1t": w1t,
        "w2t": w2t,
        "wrt": wrt,
        "brow": np.ascontiguousarray(b_r[None, :]),
        "a1t": a1t,
        "a2t": a2t,
        "b1s": b1s,
        "b2sc": b2sc,
        "b2t": b2t,
        "b1r": b1r,
        "ident": np.eye(128, dtype=np.float32),
        "ones": np.ones((1, 128), dtype=np.float32),
        "onesb": np.ones((1, 128), dtype=bf),
        "eoer": (np.arange(ER, dtype=np.float32) // R).reshape(ER, 1),
    }
    xf = x.reshape(N, D)
    in_maps = []
    for c in range(N_CORES):
        xc = xf[c * NT:(c + 1) * NT]
        xt = np.ascontiguousarray(xc.reshape(NT, DC, 128).transpose(2, 1, 0))
        in_maps.append({"xt": xt, "xb": xt.astype(bf), **shared})
    cfg = {"D": D, "F": F, "E": E, "R": R, "NT": NT}
    return in_maps, cfg, (B_, S_, N)


_nc_cache = {}


def _run(inputs, trace=False, trace_cores=None, debug=False):
    in_maps, cfg, (B_, S_, N) = _host_prep(**inputs)
    if debug:
        cfg["debug"] = True
    key = tuple(sorted(cfg.items()))
    if key not in _nc_cache:
        _nc_cache[key] = _build(cfg)
    nc = _nc_cache[key]
    res = bass_utils.run_bass_kernel_spmd(
        nc,
        in_maps,
        core_ids=list(range(N_CORES)),
        trace=trace,
        trace_cores=trace_cores,
    )
    D, NT = cfg["D"], cfg["NT"]
    out = np.empty((N, D), dtype=np.float32)
    for c in range(N_CORES):
        out[c * NT:(c + 1) * NT] = res.results[c]["rest"].T
    return out.reshape(B_, S_, D), res


def kernel(**inputs):
    out, _ = _run(inputs)
    return out
